# revision 48
# baseline (speedup 1.0000x reference)
"""Trainium2 Bass kernel for nn_Attention_22600117911625.

Multi-head causal attention with interleaved RoPE:
  out = softmax(mask(RoPE(xWq^T) RoPE(xWk^T)^T / sqrt(128))) (xWv^T) Wo^T

Sharding over 8 NeuronCores: data-parallel over batch (2) x tensor-parallel
over 4 head-groups (4 heads each).  Per core:
  phase 1: Q^T/K^T (head-dim-major, de-interleave-permuted) + V projections
           from x^T, RoPE applied via a swap-matmul + cos/sin tables,
           spilled to DRAM.
  phase 2: per head, transposed flash-style causal attention:
           S^T chunks = K^T_chunk^T Q^T, exp on ScalarE, row-sums via a
           ones-matmul, PV accumulated in PSUM, normalized A^T.
  phase 3: AllGather A^T across the 4-core group (per head, overlapped),
           then each core computes its s-quarter of out^T = Wo A^T.
Host side only reshapes/slices inputs and concatenates/transposes outputs.

All matmuls run in float32r (full-rate fp32, ~1.5e-4 rel err); softmax in
fp32 on the Scalar/Vector engines.
"""
import math

import numpy as np

import concourse.bass as bass
import concourse.mybir as mybir
from concourse import bass2jax
from concourse.tile import TileContext
from concourse.vector_clock import ScopedClock

F32 = mybir.dt.float32
F32R = mybir.dt.float32r
AF = mybir.ActivationFunctionType

B = 2
S = 4096
DM = 2048
H = 16
DH = 128
N_CORES = 8
GROUPS = 4          # tensor-parallel head groups
HL = H // GROUPS    # heads per core (4)
EL = HL * DH        # local head width (512)
SB = 512            # s-block width
NSB = S // SB       # 8
ECH = DM // 128     # 16 e-chunks
NKC = S // 128      # 32 k-chunks
SCALE = 1.0 / math.sqrt(DH)
MASK_NEG = -3.0e8
REPLICA_GROUPS = [[0, 1, 2, 3], [4, 5, 6, 7]]

_wsplit_cnt = [0]


class TC(TileContext):
    """TileContext for a walrus build that allows only ONE semaphore wait per
    instruction: extra waits are split onto nofuse NOPs on the same engine."""

    def _lower_ordered_insts(self, ordered):
        for bb_name in list(ordered.keys()):
            new = []
            for inst in ordered[bb_name]:
                si = getattr(inst, "sync_info", None)
                if si is not None and len(si.on_wait) > 1:
                    waits = list(si.on_wait)
                    eng = getattr(inst, "engine", None)
                    if eng is not None:
                        for w in waits[:-1]:
                            _wsplit_cnt[0] += 1
                            new.append(mybir.InstNoOp(
                                name=f"wsplit{_wsplit_cnt[0]}",
                                sync_info=mybir.SyncInfo(on_wait=[w], on_update=[]),
                                bass_nofuse=True,
                                engine=eng,
                            ))
                        inst.sync_info = mybir.SyncInfo(
                            on_wait=[waits[-1]], on_update=list(si.on_update))
                new.append(inst)
            ordered[bb_name] = new
        super()._lower_ordered_insts(ordered)

    def _drain_and_barrier(self, tick_clock, wait_clock):
        probe = self.nc.sync.nop(nofuse=True, hint="drain_wait_probe")
        probe.ins.sync_info = mybir.SyncInfo(on_wait=[], on_update=[])
        wait_clock.add_sem_waits(probe.ins, ScopedClock({None: tick_clock.global_clock}))
        waits = list(probe.ins.sync_info.on_wait)
        probe.ins.sync_info = mybir.SyncInfo(on_wait=waits[:1], on_update=[])
        for w in waits[1:]:
            n = self.nc.sync.nop(nofuse=True, hint="drain_wait_split")
            n.ins.sync_info = mybir.SyncInfo(on_wait=[w], on_update=[])
        self.nc.sync.drain()
        self.nc.all_engine_barrier()
        popped = self.nc._tile_sem_poison_stack.pop()
        assert popped is self._sem_poison
        self.nc.clear_and_free_semaphores(list(self.sems.allocated().values()))
        self.nc.all_engine_barrier()


def build_nc(phases=(1, 2, 3), dump=()):
    """Build the per-core SPMD kernel.  dump: subset of {"qkv", "at"} adds
    debug ExternalOutputs copied from the DRAM spill buffers."""
    nc = bass.Bass()

    xT = nc.declare_dram_parameter("xT", [DM, S], F32R, isOutput=False)
    wq = nc.declare_dram_parameter("wq", [128, ECH * EL], F32R, isOutput=False)
    wk = nc.declare_dram_parameter("wk", [128, ECH * EL], F32R, isOutput=False)
    wv = nc.declare_dram_parameter("wv", [128, ECH * EL], F32R, isOutput=False)
    wo = nc.declare_dram_parameter("wo", [HL, 128, DM], F32R, isOutput=False)
    cosT = nc.declare_dram_parameter("cosT", [128, S], F32, isOutput=False)
    sinT = nc.declare_dram_parameter("sinT", [128, S], F32, isOutput=False)
    swapM = nc.declare_dram_parameter("swapM", [128, 128], F32R, isOutput=False)
    onesW = nc.declare_dram_parameter("onesW", [128, 128], F32R, isOutput=False)
    masks = nc.declare_dram_parameter("masks", [4, 128, SB], F32, isOutput=False)
    # phase 3 is sharded over d_model: this core computes out^T rows for its
    # group's 512 d_model columns (selected host-side via the wo slice).
    outT = nc.declare_dram_parameter("outT", [EL, S], F32, isOutput=True)

    with TC(nc) as tc:
        with (
            tc.tile_pool(name="const", bufs=1) as constp,
            tc.tile_pool(name="dram", bufs=1, space="DRAM") as dram,
        ):
            ones_sb = constp.tile([128, 128], F32R)
            nc.sync.dma_start(out=ones_sb[:], in_=onesW[:])

            qT_d = [dram.tile([128, S], F32R, name=f"qT_d{h}") for h in range(HL)]
            kT_d = [dram.tile([128, S], F32R, name=f"kT_d{h}") for h in range(HL)]
            v_d = [dram.tile([128, S], F32R, name=f"v_d{h}") for h in range(HL)]
            # A^T gather segments: (start, width) in s-columns; the last
            # head uses finer segments so the final AllGather tail is small.
            seg_layout = [[(0, 2048), (2048, 2048)]] * (HL - 1) + [
                [(0, 2048), (2048, 1024), (3072, 1024)]]
            aT_l = [[dram.tile([128, w], F32R, name=f"aT_l{h}_{si}")
                     for si, (st, w) in enumerate(seg_layout[h])]
                    for h in range(HL)]
            aT_f = [[dram.tile([GROUPS * 128, w], F32R, name=f"aT_f{h}_{si}")
                     for si, (st, w) in enumerate(seg_layout[h])]
                    for h in range(HL)]

            if 1 in phases:
                _phase1(nc, tc, xT, wq, wk, wv, cosT, sinT, swapM,
                        qT_d, kT_d, v_d)
            if 2 in phases:
                _phase2(nc, tc, ones_sb, masks, qT_d, kT_d, v_d,
                        aT_l, aT_f, seg_layout)
            if 3 in phases:
                _phase3(nc, tc, wo, aT_f, seg_layout, outT)

            if "qkv" in dump:
                for nm, bufs in (("qdump", qT_d), ("kdump", kT_d), ("vdump", v_d)):
                    out = nc.declare_dram_parameter(nm, [HL, 128, S], F32R, isOutput=True)
                    for h in range(HL):
                        nc.sync.dma_start(out=out[h], in_=bufs[h][:])
            if "at" in dump:
                out = nc.declare_dram_parameter("atdump", [HL, 128, S], F32R, isOutput=True)
                for h in range(HL):
                    for si, (st, w) in enumerate(seg_layout[h]):
                        nc.sync.dma_start(out=out[h][:, st:st + w],
                                          in_=aT_l[h][si][:])
    return nc


def _phase1(nc, tc, xT, wq, wk, wv, cosT, sinT, swapM, qT_d, kT_d, v_d):
    with (
        tc.tile_pool(name="p1w", bufs=1) as wpool,
        tc.tile_pool(name="p1x", bufs=2) as xpool,
        tc.tile_pool(name="p1wv", bufs=1) as wvpool,
        tc.tile_pool(name="p1st", bufs=2) as stage,
        tc.tile_pool(name="p1ps", bufs=3, space="PSUM") as pspool,
        tc.tile_pool(name="p1psv", bufs=2, space="PSUM") as psvpool,
        tc.tile_pool(name="p1pssw", bufs=2, space="PSUM") as psswap,
    ):
        wq_sb = wpool.tile([128, ECH * EL], F32R)
        nc.sync.dma_start(out=wq_sb[:], in_=wq[:])
        wk_sb = wpool.tile([128, ECH * EL], F32R)
        nc.sync.dma_start(out=wk_sb[:], in_=wk[:])
        cos_sb = wpool.tile([128, S], F32)
        nc.sync.dma_start(out=cos_sb[:], in_=cosT[:])
        sin_sb = wpool.tile([128, S], F32)
        nc.sync.dma_start(out=sin_sb[:], in_=sinT[:])
        swap_sb = wpool.tile([128, 128], F32R)
        nc.sync.dma_start(out=swap_sb[:], in_=swapM[:])

        xT_r = xT[:].rearrange("(ec p) s -> p ec s", p=128)
        for sb in range(NSB):
            xt = xpool.tile([128, ECH * SB], F32R, tag="xt")
            nc.sync.dma_start(
                out=xt[:].rearrange("p (ec s) -> p ec s", ec=ECH),
                in_=xT_r[:, :, sb * SB:(sb + 1) * SB])

            # Q^T and K^T head-tiles, accumulated over e-chunks, then RoPE
            for wsb, dst in ((wq_sb, qT_d), (wk_sb, kT_d)):
                for h in range(HL):
                    ps = pspool.tile([128, SB], F32, tag="proj")
                    for ec in range(ECH):
                        nc.tensor.matmul(
                            ps[:],
                            wsb[:, ec * EL + h * 128: ec * EL + (h + 1) * 128],
                            xt[:, ec * SB:(ec + 1) * SB],
                            start=(ec == 0), stop=(ec == ECH - 1))
                    raw = stage.tile([128, SB], F32R, tag="raw")
                    nc.scalar.copy(raw[:], ps[:])
                    ps_sw = psswap.tile([128, SB], F32, tag="swap")
                    nc.tensor.matmul(ps_sw[:], swap_sb[:], raw[:],
                                     start=True, stop=True)
                    # in-place: raw *= cos ; ps_sw *= sin ; rot = raw + ps_sw
                    nc.vector.tensor_mul(raw[:], raw[:],
                                         cos_sb[:, sb * SB:(sb + 1) * SB])
                    nc.vector.tensor_mul(ps_sw[:], ps_sw[:],
                                         sin_sb[:, sb * SB:(sb + 1) * SB])
                    rot = stage.tile([128, SB], F32R, tag="rot")
                    nc.vector.tensor_add(rot[:], raw[:], ps_sw[:])
                    nc.sync.dma_start(out=dst[h][:, sb * SB:(sb + 1) * SB],
                                      in_=rot[:])

            # V natural layout: lhsT = x^T chunk slice (stationary), rhs = wv
            wvt = wvpool.tile([128, ECH * EL], F32R, tag="wv")
            nc.sync.dma_start(out=wvt[:], in_=wv[:])
            for st in range(4):
                psv = psvpool.tile([128, SB], F32, tag="projv")
                for ec in range(ECH):
                    nc.tensor.matmul(
                        psv[:],
                        xt[:, ec * SB + st * 128: ec * SB + (st + 1) * 128],
                        wvt[:, ec * EL:(ec + 1) * EL],
                        start=(ec == 0), stop=(ec == ECH - 1))
                vst = stage.tile([128, EL], F32R, tag="vst")
                nc.scalar.copy(vst[:], psv[:])
                stg = sb * 4 + st
                for h in range(HL):
                    nc.sync.dma_start(
                        out=v_d[h][:, stg * 128:(stg + 1) * 128],
                        in_=vst[:, h * 128:(h + 1) * 128])


def _phase2(nc, tc, ones_sb, masks, qT_d, kT_d, v_d, aT_l, aT_f, seg_layout):
    with (
        tc.tile_pool(name="p2kv", bufs=2) as kvpool,
        tc.tile_pool(name="p2pt", bufs=3) as ptpool,
        tc.tile_pool(name="p2st", bufs=3) as stage,
        tc.tile_pool(name="p2pss", bufs=2, space="PSUM") as pss,
        tc.tile_pool(name="p2psa", bufs=1, space="PSUM") as psa,
        tc.tile_pool(name="p2psl", bufs=1, space="PSUM") as psl,
    ):
        masks_sb = stage.tile([128, 4 * SB], F32, bufs=1, name="masks_sb")
        for p in range(4):
            nc.sync.dma_start(out=masks_sb[:, p * SB:(p + 1) * SB], in_=masks[p])
        for h in range(HL):
            kt = kvpool.tile([128, S], F32R, tag="kt")
            nc.sync.dma_start(out=kt[:], in_=kT_d[h][:])
            qt = kvpool.tile([128, S], F32R, tag="qt")
            nc.sync.dma_start(out=qt[:], in_=qT_d[h][:])
            vt = kvpool.tile([128, S], F32R, tag="vt")
            nc.sync.dma_start(out=vt[:], in_=v_d[h][:])
            for j in range(NSB):
                ps_a = psa.tile([128, SB], F32, tag="a")
                ps_l = psl.tile([128, SB], F32, tag="l")
                nk = 4 * j + 4
                npair = nk // 2
                for pr in range(npair):
                    # two k-chunks share one 1024-wide PSUM tile so the exp
                    # runs once per pair (amortizes ACT fixed overhead)
                    ps_s = pss.tile([128, 2 * SB], F32, tag="s")
                    pt = ptpool.tile([128, 2 * SB], F32R, tag="pt")
                    for half in range(2):
                        kc = 2 * pr + half
                        # causal: columns below p*128 of a diagonal chunk are
                        # fully masked; skip them in every consumer
                        off = max(0, (kc - 4 * j) * 128) if kc >= 4 * j else 0
                        sl = slice(half * SB + off, (half + 1) * SB)
                        nc.tensor.matmul(ps_s[:, sl],
                                         kt[:, kc * 128:(kc + 1) * 128],
                                         qt[:, j * SB + off:(j + 1) * SB],
                                         start=True, stop=True)
                        if kc >= 4 * j:
                            p = kc - 4 * j
                            nc.vector.tensor_add(
                                ps_s[:, sl], ps_s[:, sl],
                                masks_sb[:, p * SB + off:(p + 1) * SB])
                    nc.scalar.activation(pt[:], ps_s[:], AF.Exp, scale=SCALE)
                    for half in range(2):
                        kc = 2 * pr + half
                        off = max(0, (kc - 4 * j) * 128) if kc >= 4 * j else 0
                        sl = slice(half * SB + off, (half + 1) * SB)
                        osl = slice(off, SB)
                        nc.tensor.matmul(ps_l[:, osl], ones_sb[:], pt[:, sl],
                                         start=(kc == 0), stop=(kc == nk - 1))
                        nc.tensor.matmul(ps_a[:, osl],
                                         vt[:, kc * 128:(kc + 1) * 128],
                                         pt[:, sl],
                                         start=(kc == 0), stop=(kc == nk - 1))
                lnl = stage.tile([128, SB], F32, tag="lnl")
                nc.scalar.activation(lnl[:], ps_l[:], AF.Ln)
                linv = stage.tile([128, SB], F32, tag="linv")
                nc.scalar.activation(linv[:], lnl[:], AF.Exp, scale=-1.0)
                at = stage.tile([128, SB], F32R, tag="at")
                nc.vector.tensor_mul(at[:], ps_a[:], linv[:])
                q0 = j * SB
                for si, (st, w) in enumerate(seg_layout[h]):
                    if st <= q0 < st + w:
                        break
                nc.sync.dma_start(
                    out=aT_l[h][si][:, q0 - st:q0 - st + SB], in_=at[:])
                if q0 + SB == st + w:
                    nc.gpsimd.collective_compute(
                        "AllGather", mybir.AluOpType.bypass,
                        replica_groups=REPLICA_GROUPS,
                        ins=[aT_l[h][si][:]],
                        outs=[aT_f[h][si][:]])


def _phase3(nc, tc, wo, aT_f, seg_layout, outT):
    # e-chunk order: heads 0..2 first, head 3 last, so the first 12 matmuls
    # of each accumulation only depend on the earlier AllGathers
    ecg_order = [r * HL + h for h in range(HL) for r in range(GROUPS)]
    with (
        tc.tile_pool(name="p3a", bufs=32) as apool,
        tc.tile_pool(name="p3w", bufs=HL) as wopool,
        tc.tile_pool(name="p3st", bufs=3) as stage,
        tc.tile_pool(name="p3ps", bufs=2, space="PSUM") as pso,
    ):
        wot = []
        for dml in range(HL):
            t = wopool.tile([128, DM], F32R, tag="wo")
            nc.sync.dma_start(out=t[:], in_=wo[dml])
            wot.append(t)
        for sb in range(NSB):
            q0 = sb * SB
            at_sb = {}
            for ecg in ecg_order:
                r, h = divmod(ecg, HL)
                for si, (st, w) in enumerate(seg_layout[h]):
                    if st <= q0 < st + w:
                        break
                t = apool.tile([128, SB], F32R, tag="atf")
                nc.sync.dma_start(
                    out=t[:],
                    in_=aT_f[h][si][r * 128:(r + 1) * 128,
                                    q0 - st:q0 - st + SB])
                at_sb[ecg] = t
            for dml in range(HL):
                ps = pso.tile([128, SB], F32, tag="o")
                for i, ecg in enumerate(ecg_order):
                    nc.tensor.matmul(
                        ps[:],
                        wot[dml][:, ecg * 128:(ecg + 1) * 128],
                        at_sb[ecg][:],
                        start=(i == 0), stop=(i == ECH - 1))
                osb = stage.tile([128, SB], F32, tag="osb")
                nc.scalar.copy(osb[:], ps[:])
                nc.sync.dma_start(
                    out=outT[dml * 128:(dml + 1) * 128, sb * SB:(sb + 1) * SB],
                    in_=osb[:])


def _host_prep(x, Wq, Wk, Wv, Wo):
    perm = np.concatenate([np.arange(0, DH, 2), np.arange(1, DH, 2)])  # evens then odds
    rowperm = np.concatenate([h * DH + perm for h in range(HL)])

    def tile_w(Wg):  # (EL, DM) -> (128, ECH*EL): [p, ec*EL+m] = Wg[m, ec*128+p]
        return np.ascontiguousarray(
            Wg.reshape(EL, ECH, 128).transpose(2, 1, 0).reshape(128, ECH * EL))

    inv_freq = (1.0 / (10000.0 ** (np.arange(0, DH, 2) / DH))).astype(np.float64)
    pos = np.arange(S, dtype=np.float64)
    freqs = np.outer(inv_freq, pos)  # (64, S)
    cosT = np.concatenate([np.cos(freqs), np.cos(freqs)], 0).astype(np.float32)
    sinT = np.concatenate([-np.sin(freqs), np.sin(freqs)], 0).astype(np.float32)

    swap = np.zeros((128, 128), np.float32)
    for m in range(128):
        swap[(m + 64) % 128, m] = 1.0
    onesW = np.ones((128, 128), np.float32)
    masks = np.zeros((4, 128, SB), np.float32)
    ki = np.arange(128)[:, None]
    qi = np.arange(SB)[None, :]
    for p in range(4):
        masks[p] = np.where(qi >= ki + p * 128, 0.0, MASK_NEG)

    in_maps = []
    for c in range(N_CORES):
        b, g = divmod(c, GROUPS)
        sl = slice(g * EL, (g + 1) * EL)
        # wo[dml, p, ec*128+m] = Wo[g*EL + dml*128 + m, ec*128 + p]
        wo_t = np.ascontiguousarray(
            Wo[sl].reshape(HL, 128, ECH, 128).transpose(0, 3, 2, 1)
            .reshape(HL, 128, DM))
        in_maps.append({
            "xT": np.ascontiguousarray(x[b].T),
            "wq": tile_w(Wq[sl][rowperm]),
            "wk": tile_w(Wk[sl][rowperm]),
            "wv": tile_w(Wv[sl]),
            "wo": wo_t,
            "cosT": cosT,
            "sinT": sinT,
            "swapM": swap,
            "onesW": onesW,
            "masks": masks,
        })
    return in_maps


def kernel(x, Wq, Wk, Wv, Wo):
    in_maps = _host_prep(x, Wq, Wk, Wv, Wo)
    nc = build_nc()
    res = bass2jax.run_bass_via_pjrt(nc, in_maps, n_cores=N_CORES)
    out = np.empty((B, S, DM), np.float32)
    for c in range(N_CORES):
        b, g = divmod(c, GROUPS)
        out[b, :, g * EL:(g + 1) * EL] = res[c]["outT"].T
    return out


if __name__ == "__main__":
    rng = np.random.default_rng(0)
    x = rng.standard_normal((B, S, DM)).astype(np.float32)
    Wq = (rng.standard_normal((H * DH, DM)) * 0.02).astype(np.float32)
    Wk = (rng.standard_normal((H * DH, DM)) * 0.02).astype(np.float32)
    Wv = (rng.standard_normal((H * DH, DM)) * 0.02).astype(np.float32)
    Wo = (rng.standard_normal((DM, H * DH)) * 0.02).astype(np.float32)
    out = kernel(x, Wq, Wk, Wv, Wo)
    print(out.shape, out.dtype)


# revision 49
# speedup vs baseline: 1.0748x; 1.0748x over previous
"""Trainium2 Bass kernel for nn_Attention_22600117911625.

Multi-head causal attention with interleaved RoPE:
  out = softmax(mask(RoPE(xWq^T) RoPE(xWk^T)^T / sqrt(128))) (xWv^T) Wo^T

Sharding over 8 NeuronCores: data-parallel over batch (2) x tensor-parallel
over 4 head-groups (4 heads each).  Per core:
  phase 1: Q^T/K^T (head-dim-major, de-interleave-permuted) + V projections
           from x^T, RoPE applied via a swap-matmul + cos/sin tables,
           spilled to DRAM.
  phase 2: per head, transposed flash-style causal attention:
           S^T chunks = K^T_chunk^T Q^T, exp on ScalarE, row-sums via a
           ones-matmul, PV accumulated in PSUM, normalized A^T.
  phase 3: AllGather A^T across the 4-core group (per head, overlapped),
           then each core computes its s-quarter of out^T = Wo A^T.
Host side only reshapes/slices inputs and concatenates/transposes outputs.

All matmuls run in float32r (full-rate fp32, ~1.5e-4 rel err); softmax in
fp32 on the Scalar/Vector engines.
"""
import math

import numpy as np

import concourse.bass as bass
import concourse.mybir as mybir
from concourse import bass2jax
from concourse.tile import TileContext
from concourse.vector_clock import ScopedClock

F32 = mybir.dt.float32
F32R = mybir.dt.float32r
AF = mybir.ActivationFunctionType

B = 2
S = 4096
DM = 2048
H = 16
DH = 128
N_CORES = 8
GROUPS = 4          # tensor-parallel head groups
HL = H // GROUPS    # heads per core (4)
EL = HL * DH        # local head width (512)
SB = 512            # s-block width
NSB = S // SB       # 8
ECH = DM // 128     # 16 e-chunks
NKC = S // 128      # 32 k-chunks
SCALE = 1.0 / math.sqrt(DH)
MASK_NEG = -3.0e8
REPLICA_GROUPS = [[0, 1, 2, 3], [4, 5, 6, 7]]

_wsplit_cnt = [0]


class TC(TileContext):
    """TileContext for a walrus build that allows only ONE semaphore wait per
    instruction: extra waits are split onto nofuse NOPs on the same engine."""

    def _lower_ordered_insts(self, ordered):
        for bb_name in list(ordered.keys()):
            new = []
            for inst in ordered[bb_name]:
                si = getattr(inst, "sync_info", None)
                if si is not None and len(si.on_wait) > 1:
                    waits = list(si.on_wait)
                    eng = getattr(inst, "engine", None)
                    if eng is not None:
                        for w in waits[:-1]:
                            _wsplit_cnt[0] += 1
                            new.append(mybir.InstNoOp(
                                name=f"wsplit{_wsplit_cnt[0]}",
                                sync_info=mybir.SyncInfo(on_wait=[w], on_update=[]),
                                bass_nofuse=True,
                                engine=eng,
                            ))
                        inst.sync_info = mybir.SyncInfo(
                            on_wait=[waits[-1]], on_update=list(si.on_update))
                new.append(inst)
            ordered[bb_name] = new
        super()._lower_ordered_insts(ordered)

    def _drain_and_barrier(self, tick_clock, wait_clock):
        probe = self.nc.sync.nop(nofuse=True, hint="drain_wait_probe")
        probe.ins.sync_info = mybir.SyncInfo(on_wait=[], on_update=[])
        wait_clock.add_sem_waits(probe.ins, ScopedClock({None: tick_clock.global_clock}))
        waits = list(probe.ins.sync_info.on_wait)
        probe.ins.sync_info = mybir.SyncInfo(on_wait=waits[:1], on_update=[])
        for w in waits[1:]:
            n = self.nc.sync.nop(nofuse=True, hint="drain_wait_split")
            n.ins.sync_info = mybir.SyncInfo(on_wait=[w], on_update=[])
        self.nc.sync.drain()
        self.nc.all_engine_barrier()
        popped = self.nc._tile_sem_poison_stack.pop()
        assert popped is self._sem_poison
        self.nc.clear_and_free_semaphores(list(self.sems.allocated().values()))
        self.nc.all_engine_barrier()


def build_nc(phases=(1, 2, 3), dump=()):
    """Build the per-core SPMD kernel.  dump: subset of {"qkv", "at"} adds
    debug ExternalOutputs copied from the DRAM spill buffers."""
    nc = bass.Bass()

    xT = nc.declare_dram_parameter("xT", [DM, S], F32R, isOutput=False)
    wq = nc.declare_dram_parameter("wq", [128, ECH * EL], F32R, isOutput=False)
    wk = nc.declare_dram_parameter("wk", [128, ECH * EL], F32R, isOutput=False)
    wv = nc.declare_dram_parameter("wv", [128, ECH * EL], F32R, isOutput=False)
    wo = nc.declare_dram_parameter("wo", [HL, 128, DM], F32R, isOutput=False)
    cosT = nc.declare_dram_parameter("cosT", [128, S], F32, isOutput=False)
    sinT = nc.declare_dram_parameter("sinT", [128, S], F32, isOutput=False)
    swapM = nc.declare_dram_parameter("swapM", [128, 128], F32R, isOutput=False)
    onesW = nc.declare_dram_parameter("onesW", [128, 128], F32R, isOutput=False)
    masks = nc.declare_dram_parameter("masks", [4, 128, SB], F32, isOutput=False)
    # phase 3 is sharded over d_model: this core computes out^T rows for its
    # group's 512 d_model columns (selected host-side via the wo slice).
    outT = nc.declare_dram_parameter("outT", [EL, S], F32, isOutput=True)

    with TC(nc) as tc:
        with (
            tc.tile_pool(name="const", bufs=1) as constp,
            tc.tile_pool(name="dram", bufs=1, space="DRAM") as dram,
        ):
            ones_sb = constp.tile([128, 128], F32R)
            nc.sync.dma_start(out=ones_sb[:], in_=onesW[:])

            qT_d = [dram.tile([128, S], F32R, name=f"qT_d{h}") for h in range(HL)]
            kT_d = [dram.tile([128, S], F32R, name=f"kT_d{h}") for h in range(HL)]
            v_d = [dram.tile([128, S], F32R, name=f"v_d{h}") for h in range(HL)]
            # A^T gather segments: (start, width) in s-columns; the last
            # head uses finer segments so the final AllGather tail is small.
            seg_layout = [[(0, 2048), (2048, 2048)]] * (HL - 1) + [
                [(0, 2048), (2048, 1024), (3072, 1024)]]
            aT_l = [[dram.tile([128, w], F32R, name=f"aT_l{h}_{si}")
                     for si, (st, w) in enumerate(seg_layout[h])]
                    for h in range(HL)]
            aT_f = [[dram.tile([GROUPS * 128, w], F32R, name=f"aT_f{h}_{si}")
                     for si, (st, w) in enumerate(seg_layout[h])]
                    for h in range(HL)]

            if 1 in phases:
                _phase1(nc, tc, xT, wq, wk, wv, cosT, sinT, swapM,
                        qT_d, kT_d, v_d)
            if 2 in phases:
                _phase2(nc, tc, ones_sb, masks, qT_d, kT_d, v_d,
                        aT_l, aT_f, seg_layout)
            if 3 in phases:
                _phase3(nc, tc, wo, aT_f, seg_layout, outT)

            if "qkv" in dump:
                for nm, bufs in (("qdump", qT_d), ("kdump", kT_d), ("vdump", v_d)):
                    out = nc.declare_dram_parameter(nm, [HL, 128, S], F32R, isOutput=True)
                    for h in range(HL):
                        nc.sync.dma_start(out=out[h], in_=bufs[h][:])
            if "at" in dump:
                out = nc.declare_dram_parameter("atdump", [HL, 128, S], F32R, isOutput=True)
                for h in range(HL):
                    for si, (st, w) in enumerate(seg_layout[h]):
                        nc.sync.dma_start(out=out[h][:, st:st + w],
                                          in_=aT_l[h][si][:])
    return nc


def _phase1(nc, tc, xT, wq, wk, wv, cosT, sinT, swapM, qT_d, kT_d, v_d):
    with (
        tc.tile_pool(name="p1w", bufs=1) as wpool,
        tc.tile_pool(name="p1x", bufs=2) as xpool,
        tc.tile_pool(name="p1wv", bufs=1) as wvpool,
        tc.tile_pool(name="p1st", bufs=2) as stage,
        tc.tile_pool(name="p1ps", bufs=3, space="PSUM") as pspool,
        tc.tile_pool(name="p1psv", bufs=2, space="PSUM") as psvpool,
        tc.tile_pool(name="p1pssw", bufs=2, space="PSUM") as psswap,
    ):
        wq_sb = wpool.tile([128, ECH * EL], F32R)
        nc.sync.dma_start(out=wq_sb[:], in_=wq[:])
        wk_sb = wpool.tile([128, ECH * EL], F32R)
        nc.sync.dma_start(out=wk_sb[:], in_=wk[:])
        cos_sb = wpool.tile([128, S], F32)
        nc.sync.dma_start(out=cos_sb[:], in_=cosT[:])
        sin_sb = wpool.tile([128, S], F32)
        nc.sync.dma_start(out=sin_sb[:], in_=sinT[:])
        swap_sb = wpool.tile([128, 128], F32R)
        nc.sync.dma_start(out=swap_sb[:], in_=swapM[:])

        xT_r = xT[:].rearrange("(ec p) s -> p ec s", p=128)
        for sb in range(NSB):
            xt = xpool.tile([128, ECH * SB], F32R, tag="xt")
            nc.sync.dma_start(
                out=xt[:].rearrange("p (ec s) -> p ec s", ec=ECH),
                in_=xT_r[:, :, sb * SB:(sb + 1) * SB])

            # Q^T and K^T head-tiles, accumulated over e-chunks, then RoPE
            for wsb, dst in ((wq_sb, qT_d), (wk_sb, kT_d)):
                for h in range(HL):
                    ps = pspool.tile([128, SB], F32, tag="proj")
                    for ec in range(ECH):
                        nc.tensor.matmul(
                            ps[:],
                            wsb[:, ec * EL + h * 128: ec * EL + (h + 1) * 128],
                            xt[:, ec * SB:(ec + 1) * SB],
                            start=(ec == 0), stop=(ec == ECH - 1))
                    raw = stage.tile([128, SB], F32R, tag="raw")
                    nc.scalar.copy(raw[:], ps[:])
                    ps_sw = psswap.tile([128, SB], F32, tag="swap")
                    nc.tensor.matmul(ps_sw[:], swap_sb[:], raw[:],
                                     start=True, stop=True)
                    # in-place: raw *= cos ; ps_sw *= sin ; rot = raw + ps_sw
                    nc.vector.tensor_mul(raw[:], raw[:],
                                         cos_sb[:, sb * SB:(sb + 1) * SB])
                    nc.vector.tensor_mul(ps_sw[:], ps_sw[:],
                                         sin_sb[:, sb * SB:(sb + 1) * SB])
                    rot = stage.tile([128, SB], F32R, tag="rot")
                    nc.vector.tensor_add(rot[:], raw[:], ps_sw[:])
                    nc.sync.dma_start(out=dst[h][:, sb * SB:(sb + 1) * SB],
                                      in_=rot[:])

            # V natural layout: lhsT = x^T chunk slice (stationary), rhs = wv
            wvt = wvpool.tile([128, ECH * EL], F32R, tag="wv")
            nc.sync.dma_start(out=wvt[:], in_=wv[:])
            for st in range(4):
                psv = psvpool.tile([128, SB], F32, tag="projv")
                for ec in range(ECH):
                    nc.tensor.matmul(
                        psv[:],
                        xt[:, ec * SB + st * 128: ec * SB + (st + 1) * 128],
                        wvt[:, ec * EL:(ec + 1) * EL],
                        start=(ec == 0), stop=(ec == ECH - 1))
                vst = stage.tile([128, EL], F32R, tag="vst")
                nc.scalar.copy(vst[:], psv[:])
                stg = sb * 4 + st
                for h in range(HL):
                    nc.sync.dma_start(
                        out=v_d[h][:, stg * 128:(stg + 1) * 128],
                        in_=vst[:, h * 128:(h + 1) * 128])


def _phase2(nc, tc, ones_sb, masks, qT_d, kT_d, v_d, aT_l, aT_f, seg_layout):
    with (
        tc.tile_pool(name="p2kv", bufs=2) as kvpool,
        tc.tile_pool(name="p2pt", bufs=3) as ptpool,
        tc.tile_pool(name="p2st", bufs=3) as stage,
        tc.tile_pool(name="p2pss", bufs=3, space="PSUM") as pss,
        tc.tile_pool(name="p2psa", bufs=1, space="PSUM") as psa,
        tc.tile_pool(name="p2psl", bufs=1, space="PSUM") as psl,
    ):
        masks_sb = stage.tile([128, 4 * SB], F32, bufs=1, name="masks_sb")
        for p in range(4):
            nc.sync.dma_start(out=masks_sb[:, p * SB:(p + 1) * SB], in_=masks[p])
        for h in range(HL):
            kt = kvpool.tile([128, S], F32R, tag="kt")
            nc.sync.dma_start(out=kt[:], in_=kT_d[h][:])
            qt = kvpool.tile([128, S], F32R, tag="qt")
            nc.sync.dma_start(out=qt[:], in_=qT_d[h][:])
            vt = kvpool.tile([128, S], F32R, tag="vt")
            nc.sync.dma_start(out=vt[:], in_=v_d[h][:])
            for j in range(NSB):
                ps_a = psa.tile([128, SB], F32, tag="a")
                ps_l = psl.tile([128, SB], F32, tag="l")
                nk = 4 * j + 4
                npair = nk // 2
                for pr in range(npair):
                    # two k-chunks share one 1024-wide PSUM tile so the exp
                    # runs once per pair (amortizes ACT fixed overhead)
                    ps_s = pss.tile([128, 2 * SB], F32, tag="s")
                    pt = ptpool.tile([128, 2 * SB], F32R, tag="pt")
                    for half in range(2):
                        kc = 2 * pr + half
                        # causal: columns below p*128 of a diagonal chunk are
                        # fully masked; skip them in every consumer
                        off = max(0, (kc - 4 * j) * 128) if kc >= 4 * j else 0
                        sl = slice(half * SB + off, (half + 1) * SB)
                        nc.tensor.matmul(ps_s[:, sl],
                                         kt[:, kc * 128:(kc + 1) * 128],
                                         qt[:, j * SB + off:(j + 1) * SB],
                                         start=True, stop=True)
                        if kc >= 4 * j:
                            p = kc - 4 * j
                            nc.vector.tensor_add(
                                ps_s[:, sl], ps_s[:, sl],
                                masks_sb[:, p * SB + off:(p + 1) * SB])
                    nc.scalar.activation(pt[:], ps_s[:], AF.Exp, scale=SCALE)
                    for half in range(2):
                        kc = 2 * pr + half
                        off = max(0, (kc - 4 * j) * 128) if kc >= 4 * j else 0
                        sl = slice(half * SB + off, (half + 1) * SB)
                        osl = slice(off, SB)
                        nc.tensor.matmul(ps_l[:, osl], ones_sb[:], pt[:, sl],
                                         start=(kc == 0), stop=(kc == nk - 1))
                        nc.tensor.matmul(ps_a[:, osl],
                                         vt[:, kc * 128:(kc + 1) * 128],
                                         pt[:, sl],
                                         start=(kc == 0), stop=(kc == nk - 1))
                lnl = stage.tile([128, SB], F32, tag="lnl")
                nc.scalar.activation(lnl[:], ps_l[:], AF.Ln)
                linv = stage.tile([128, SB], F32, tag="linv")
                nc.scalar.activation(linv[:], lnl[:], AF.Exp, scale=-1.0)
                at = stage.tile([128, SB], F32R, tag="at")
                nc.vector.tensor_mul(at[:], ps_a[:], linv[:])
                q0 = j * SB
                for si, (st, w) in enumerate(seg_layout[h]):
                    if st <= q0 < st + w:
                        break
                nc.sync.dma_start(
                    out=aT_l[h][si][:, q0 - st:q0 - st + SB], in_=at[:])
                if q0 + SB == st + w:
                    nc.gpsimd.collective_compute(
                        "AllGather", mybir.AluOpType.bypass,
                        replica_groups=REPLICA_GROUPS,
                        ins=[aT_l[h][si][:]],
                        outs=[aT_f[h][si][:]])


def _phase3(nc, tc, wo, aT_f, seg_layout, outT):
    # e-chunk order: heads 0..2 first, head 3 last, so the first 12 matmuls
    # of each accumulation only depend on the earlier AllGathers
    ecg_order = [r * HL + h for h in range(HL) for r in range(GROUPS)]
    with (
        tc.tile_pool(name="p3a", bufs=32) as apool,
        tc.tile_pool(name="p3w", bufs=HL) as wopool,
        tc.tile_pool(name="p3st", bufs=3) as stage,
        tc.tile_pool(name="p3ps", bufs=2, space="PSUM") as pso,
    ):
        wot = []
        for dml in range(HL):
            t = wopool.tile([128, DM], F32R, tag="wo")
            nc.sync.dma_start(out=t[:], in_=wo[dml])
            wot.append(t)
        for sb in range(NSB):
            q0 = sb * SB
            at_sb = {}
            for ecg in ecg_order:
                r, h = divmod(ecg, HL)
                for si, (st, w) in enumerate(seg_layout[h]):
                    if st <= q0 < st + w:
                        break
                t = apool.tile([128, SB], F32R, tag="atf")
                nc.sync.dma_start(
                    out=t[:],
                    in_=aT_f[h][si][r * 128:(r + 1) * 128,
                                    q0 - st:q0 - st + SB])
                at_sb[ecg] = t
            for dml in range(HL):
                ps = pso.tile([128, SB], F32, tag="o")
                for i, ecg in enumerate(ecg_order):
                    nc.tensor.matmul(
                        ps[:],
                        wot[dml][:, ecg * 128:(ecg + 1) * 128],
                        at_sb[ecg][:],
                        start=(i == 0), stop=(i == ECH - 1))
                osb = stage.tile([128, SB], F32, tag="osb")
                nc.scalar.copy(osb[:], ps[:])
                nc.sync.dma_start(
                    out=outT[dml * 128:(dml + 1) * 128, sb * SB:(sb + 1) * SB],
                    in_=osb[:])


def _host_prep(x, Wq, Wk, Wv, Wo):
    perm = np.concatenate([np.arange(0, DH, 2), np.arange(1, DH, 2)])  # evens then odds
    rowperm = np.concatenate([h * DH + perm for h in range(HL)])

    def tile_w(Wg):  # (EL, DM) -> (128, ECH*EL): [p, ec*EL+m] = Wg[m, ec*128+p]
        return np.ascontiguousarray(
            Wg.reshape(EL, ECH, 128).transpose(2, 1, 0).reshape(128, ECH * EL))

    inv_freq = (1.0 / (10000.0 ** (np.arange(0, DH, 2) / DH))).astype(np.float64)
    pos = np.arange(S, dtype=np.float64)
    freqs = np.outer(inv_freq, pos)  # (64, S)
    cosT = np.concatenate([np.cos(freqs), np.cos(freqs)], 0).astype(np.float32)
    sinT = np.concatenate([-np.sin(freqs), np.sin(freqs)], 0).astype(np.float32)

    swap = np.zeros((128, 128), np.float32)
    for m in range(128):
        swap[(m + 64) % 128, m] = 1.0
    onesW = np.ones((128, 128), np.float32)
    masks = np.zeros((4, 128, SB), np.float32)
    ki = np.arange(128)[:, None]
    qi = np.arange(SB)[None, :]
    for p in range(4):
        masks[p] = np.where(qi >= ki + p * 128, 0.0, MASK_NEG)

    in_maps = []
    for c in range(N_CORES):
        b, g = divmod(c, GROUPS)
        sl = slice(g * EL, (g + 1) * EL)
        # wo[dml, p, ec*128+m] = Wo[g*EL + dml*128 + m, ec*128 + p]
        wo_t = np.ascontiguousarray(
            Wo[sl].reshape(HL, 128, ECH, 128).transpose(0, 3, 2, 1)
            .reshape(HL, 128, DM))
        in_maps.append({
            "xT": np.ascontiguousarray(x[b].T),
            "wq": tile_w(Wq[sl][rowperm]),
            "wk": tile_w(Wk[sl][rowperm]),
            "wv": tile_w(Wv[sl]),
            "wo": wo_t,
            "cosT": cosT,
            "sinT": sinT,
            "swapM": swap,
            "onesW": onesW,
            "masks": masks,
        })
    return in_maps


def kernel(x, Wq, Wk, Wv, Wo):
    in_maps = _host_prep(x, Wq, Wk, Wv, Wo)
    nc = build_nc()
    res = bass2jax.run_bass_via_pjrt(nc, in_maps, n_cores=N_CORES)
    out = np.empty((B, S, DM), np.float32)
    for c in range(N_CORES):
        b, g = divmod(c, GROUPS)
        out[b, :, g * EL:(g + 1) * EL] = res[c]["outT"].T
    return out


if __name__ == "__main__":
    rng = np.random.default_rng(0)
    x = rng.standard_normal((B, S, DM)).astype(np.float32)
    Wq = (rng.standard_normal((H * DH, DM)) * 0.02).astype(np.float32)
    Wk = (rng.standard_normal((H * DH, DM)) * 0.02).astype(np.float32)
    Wv = (rng.standard_normal((H * DH, DM)) * 0.02).astype(np.float32)
    Wo = (rng.standard_normal((DM, H * DH)) * 0.02).astype(np.float32)
    out = kernel(x, Wq, Wk, Wv, Wo)
    print(out.shape, out.dtype)
